# revision 41
# baseline (speedup 1.0000x reference)
"""Multi-head causal attention (QKV proj + attention + O proj) on 8 TRN2 cores.

Sharding: data-parallel over batch (4) x tensor-parallel over heads (2 groups
of 8 heads).  Core c handles batch c//2, head-group c%2.  Each core computes
its group's partial o_proj output; the host sums the two partials per batch.

Layout strategy (all activations arrive pre-transposed from the host, so the
kernel never transposes on-device):
  - qT, kT per head-pair M-tile: (128 head-dims, L) from  W.T-slice @ X.T
  - v natural (tokens, head-dims) with a fused ones-column for the softmax
    denominator: av_psum = v_aug.T @ P.T gives (65, Nq) where row 64 is the
    per-query sum of probabilities.
  - scores are computed transposed (keys on partitions, queries free), exp is
    taken without max-subtraction (scores are O(+-6) here, exp is safe in
    fp32), causal masking multiplies a small triangular mask after exp; fully
    masked column ranges are skipped by exp.

Schedule: the scalar engine (exp) is the scarce resource in the attention
phase and its natural per-query-chunk load grows linearly with the causal
width (18/37/55/73 us per wave).  To flatten it, query-chunk 3's first
key-group is pulled forward into wave 1 (its K/V/Q inputs are ready by
then), giving per-wave exp loads of 18/55/55/55 us.  Because the tensor
engine executes matmuls strictly in order, projection chains are split into
4-matmul fragments and a fill queue pops one fragment between each score
pair and its exp-dependent AV matmul, so the ~1 us exp latency is always
covered by queued independent work.  o_proj for chunk c is emitted during
wave c+1 as fill; chunk 3's chains are split around the last softmax
normalize (kt 0-2 before, kt 3 after) and borrow the then-idle score-PSUM
pool so four chains are in flight.  The last normalize runs on-chip per
128-query block (reciprocal of denominator rows that an extra block of 64
ones-columns in the V tiles replicates across partitions); all other
normalizes use a DMA transpose through DRAM so the reciprocal runs at 4
elements/lane.  Inputs load as two-half 3D-AP DMAs (region-tracked, so
consumers start after the first half).  Outputs are stored bf16.
Compute dtype bf16 (fp32 PSUM accumulation).
"""

import numpy as np
import ml_dtypes

import concourse.bass as bass
import concourse.tile as tile
from concourse import bacc, mybir

D_MODEL = 1024
N_HEADS = 16
D_K = 64
B, L = 4, 2048
TP = 2                  # head groups
GD = D_MODEL // TP      # 512 head-dims per group
P = 128
NQ = 512                # query chunk (one fp32 PSUM bank)
N_MT = GD // P          # 4 M-tiles (head pairs) per group
N_KT = D_MODEL // P     # 8 contraction tiles over model dim
N_TT = L // P           # 16 token tiles
N_QC = L // NQ          # 4 query chunks
BF16 = mybir.dt.bfloat16
F32 = mybir.dt.float32
NPBF16 = ml_dtypes.bfloat16
AF = mybir.ActivationFunctionType
ALU = mybir.AluOpType


def build_nc() -> bass.Bass:
    nc = bacc.Bacc("TRN2", target_bir_lowering=False)

    xqT = nc.dram_tensor("xqT", [D_MODEL, L], BF16, kind="ExternalInput")
    xkT = nc.dram_tensor("xkT", [D_MODEL, L], BF16, kind="ExternalInput")
    xvT = nc.dram_tensor("xvT", [D_MODEL, L], BF16, kind="ExternalInput")
    wqT = nc.dram_tensor("wqT", [D_MODEL, GD], BF16, kind="ExternalInput")
    wkT = nc.dram_tensor("wkT", [D_MODEL, GD], BF16, kind="ExternalInput")
    wvT = nc.dram_tensor("wvT", [D_MODEL, GD], BF16, kind="ExternalInput")
    woT = nc.dram_tensor("woT", [GD, D_MODEL], BF16, kind="ExternalInput")
    bq = nc.dram_tensor("bq", [P, N_MT], F32, kind="ExternalInput")
    bk = nc.dram_tensor("bk", [P, N_MT], F32, kind="ExternalInput")
    bv = nc.dram_tensor("bv", [1, GD], F32, kind="ExternalInput")
    maskc = nc.dram_tensor("maskc", [P, P], BF16, kind="ExternalInput")
    out = nc.dram_tensor("out", [L, D_MODEL], BF16, kind="ExternalOutput")

    with tile.TileContext(nc) as tc:
        with (
            tc.tile_pool(name="const", bufs=1) as const,
            tc.tile_pool(name="xvg", bufs=2) as xvg_pool,
            tc.tile_pool(name="xch", bufs=4) as xch_pool,
            tc.tile_pool(name="pt", bufs=4) as pt_pool,
            tc.tile_pool(name="small", bufs=3) as small_pool,
            tc.tile_pool(name="acc", bufs=4) as acc_pool,
            tc.tile_pool(name="osb", bufs=4) as osb_pool,
            tc.tile_pool(name="dr", bufs=4, space="DRAM") as dr_pool,
            tc.tile_pool(name="ps_s", bufs=2, space="PSUM") as ps_s,
            tc.tile_pool(name="ps_av", bufs=2, space="PSUM") as ps_av,
            tc.tile_pool(name="ps_mm", bufs=2, space="PSUM") as ps_mm,
        ):
            # ---- resident constants / weights; first-needed tiles (wk + xk
            # chunk 0) round-robin over all three DMA queues
            QS = [nc.gpsimd, nc.scalar, nc.sync]
            wk_sb = const.tile([P, N_KT, GD], BF16, tag="wk")
            nc.gpsimd.dma_start(
                out=wk_sb[:, 0:4, :],
                in_=wkT.rearrange("(kt p) n -> p kt n", p=P)[:, 0:4, :])
            nc.scalar.dma_start(
                out=wk_sb[:, 4:8, :],
                in_=wkT.rearrange("(kt p) n -> p kt n", p=P)[:, 4:8, :])

            # per-(mt, chunk) tiles so consumers unblock as soon as possible
            qTt = [[const.tile([P, NQ], BF16, tag=f"qT{mt}_{ncz}", name=f"qT{mt}_{ncz}")
                    for ncz in range(N_QC)] for mt in range(N_MT)]
            kTt = [[const.tile([P, NQ], BF16, tag=f"kT{mt}_{ncz}", name=f"kT{mt}_{ncz}")
                    for ncz in range(N_QC)] for mt in range(N_MT)]
            vA = [const.tile([P, 2 * N_MT, 2 * D_K], BF16, tag=f"v{tt}", name=f"v{tt}")
                  for tt in range(N_TT)]
            aoTq = [[const.tile([P, NQ], BF16, tag=f"ao{mt}_{qc}", name=f"ao{mt}_{qc}")
                     for qc in range(N_QC)] for mt in range(N_MT)]
            # persistent accumulators for the pulled-forward qc=3 groups
            acc3 = [[const.tile([P, NQ], F32, tag=f"acc3_{mt}_{h2}",
                                name=f"acc3_{mt}_{h2}") for h2 in range(2)]
                    for mt in range(N_MT)]

            scale = float(1.0 / np.sqrt(np.float32(D_K)))

            wv_sb = const.tile([P, N_KT, GD], BF16, tag="wv")
            wq_sb = const.tile([P, N_KT, GD], BF16, tag="wq")
            wo_sb = const.tile([P, N_MT, D_MODEL], BF16, tag="wo")

            def late_loads():
                # per-kt loads so the scalar queue is clear before exps start;
                # wo (needed only in wave 1) rides gpsimd/sync
                for kt in range(N_KT):
                    nc.scalar.dma_start(out=wv_sb[:, kt, :],
                                        in_=wvT[kt * P:(kt + 1) * P, :])
                for kt in range(N_KT):
                    nc.scalar.dma_start(out=wq_sb[:, kt, :],
                                        in_=wqT[kt * P:(kt + 1) * P, :])
                for kt in range(N_MT):
                    eng = nc.gpsimd if kt % 2 == 0 else nc.sync
                    eng.dma_start(out=wo_sb[:, kt, :],
                                  in_=woT[kt * P:(kt + 1) * P, :])

            # ---- V projection: per-group (4 token tiles) input chunks
            xvg_cache = {}

            def xv_dmas(g, both_gpsimd=False):
                xc = xvg_pool.tile([P, N_KT, 4 * P], BF16, tag="xvg",
                                   name=f"xv{g}")
                src = xvT.rearrange("(kt p) n -> p kt n", p=P)
                e1 = nc.gpsimd if both_gpsimd else nc.sync
                nc.gpsimd.dma_start(
                    out=xc[:, 0:4, :],
                    in_=src[:, 0:4, g * 4 * P:(g + 1) * 4 * P])
                e1.dma_start(
                    out=xc[:, 4:8, :],
                    in_=src[:, 4:8, g * 4 * P:(g + 1) * 4 * P])
                xvg_cache[g] = xc

            def v_frags(tt):
                g, j = tt // 4, tt % 4
                cell = {}

                def f1():
                    xvs = [xvg_cache[g][:, kt, :] for kt in range(N_KT)]
                    ps = cell["ps"] = ps_mm.tile([P, GD], F32, tag="mm",
                                                 name=f"psv{tt}")
                    for kt in range(4):
                        nc.tensor.matmul(
                            ps, lhsT=xvs[kt][:, j * P:(j + 1) * P],
                            rhs=wv_sb[:, kt, :], start=(kt == 0), stop=False)

                def f2():
                    xvs = [xvg_cache[g][:, kt, :] for kt in range(N_KT)]
                    ps = cell["ps"]
                    for kt in range(4, N_KT):
                        nc.tensor.matmul(
                            ps, lhsT=xvs[kt][:, j * P:(j + 1) * P],
                            rhs=wv_sb[:, kt, :], start=False,
                            stop=(kt == N_KT - 1))
                    nc.vector.tensor_tensor(
                        out=vA[tt][:, :, 0:D_K],
                        in0=ps.rearrange("p (h d) -> p h d", d=D_K),
                        in1=bv_sb.rearrange("p (h d) -> p h d", d=D_K),
                        op=ALU.add,
                    )
                    nc.vector.memset(vA[tt][:, :, D_K:2 * D_K], 1.0)
                return [f1, f2]

            def v_proj_tile(tt):
                for f in v_frags(tt):
                    f()

            xch_cache = {}

            def kq_dmas(nm, x_dram, ncz, split=False):
                xc = xch_pool.tile([P, N_KT, NQ], BF16, tag="xch",
                                   name=f"x{nm}{ncz}")
                e0, e1 = (nc.sync, nc.gpsimd) if split else (nc.gpsimd, nc.sync)
                src = x_dram.rearrange("(kt p) n -> p kt n", p=P)
                e0.dma_start(
                    out=xc[:, 0:4, :],
                    in_=src[:, 0:4, ncz * NQ:(ncz + 1) * NQ])
                e1.dma_start(
                    out=xc[:, 4:8, :],
                    in_=src[:, 4:8, ncz * NQ:(ncz + 1) * NQ])
                xch_cache[(nm, ncz)] = xc

            def kq_frags(w_sb, b_sb, dsts, sc, nm, ncz, mt, act=False):
                cell = {}

                def f1():
                    xch = xch_cache[(nm, ncz)]
                    ps = cell["ps"] = ps_mm.tile([P, NQ], F32, tag="mm",
                                                 name=f"ps{nm}{ncz}{mt}")
                    for kt in range(4):
                        nc.tensor.matmul(
                            ps, lhsT=w_sb[:, kt, mt * P:(mt + 1) * P],
                            rhs=xch[:, kt, :], start=(kt == 0), stop=False)

                def f2():
                    xch = xch_cache[(nm, ncz)]
                    ps = cell["ps"]
                    for kt in range(4, N_KT):
                        nc.tensor.matmul(
                            ps, lhsT=w_sb[:, kt, mt * P:(mt + 1) * P],
                            rhs=xch[:, kt, :], start=False,
                            stop=(kt == N_KT - 1))
                    if act:  # ACT has slack outside wave 2; keeps DVE clear
                        nc.scalar.add(out=dsts[mt][ncz], in_=ps,
                                      add=b_sb[:, mt:mt + 1])
                    else:
                        nc.vector.tensor_scalar(
                            out=dsts[mt][ncz],
                            in0=ps,
                            scalar1=b_sb[:, mt:mt + 1],
                            scalar2=sc,
                            op0=ALU.add,
                            op1=ALU.mult,
                        )
                return [f1, f2]

            def kq_part(w_sb, b_sb, dsts, sc, nm, ncz, mt):
                for f in kq_frags(w_sb, b_sb, dsts, sc, nm, ncz, mt):
                    f()

            # ---- attention: one 4-key-block group (512 keys) of (mt, qc) ----
            def emit_fill(fq, n=1):
                for _ in range(n):
                    if fq:
                        fq.pop(0)()

            def attn_group(mt, qc, kb_lo, kb_hi, acc, first, fq=None):
                av = [ps_av.tile([P, NQ], F32, tag="av",
                                 name=f"av{mt}_{qc}_{kb_lo}_{i}") for i in range(2)]
                for kb in range(kb_lo, kb_hi):
                    t = P * (kb - 4 * qc)  # <0 for full blocks
                    s_ps = ps_s.tile([P, 2 * NQ], F32, tag="s",
                                     name=f"s{mt}_{qc}_{kb}")
                    s3 = s_ps.rearrange("p (h n) -> p h n", n=NQ)
                    for h2 in range(2):
                        nc.tensor.matmul(
                            s3[:, h2, max(t, 0):NQ],
                            lhsT=kTt[mt][kb // 4][h2 * D_K:(h2 + 1) * D_K,
                                                 (kb % 4) * P:(kb % 4 + 1) * P],
                            rhs=qTt[mt][qc][h2 * D_K:(h2 + 1) * D_K,
                                            max(t, 0):NQ],
                            start=True,
                            stop=True,
                        )
                    pt = pt_pool.tile([P, 2 * NQ], BF16, tag="pt",
                                      name=f"pt{mt}_{qc}_{kb}")
                    p3 = pt.rearrange("p (h n) -> p h n", n=NQ)
                    if t <= 0:
                        nc.scalar.activation(out=pt, in_=s_ps, func=AF.Exp)
                    else:
                        nc.scalar.activation(out=p3[:, :, t:NQ],
                                             in_=s3[:, :, t:NQ], func=AF.Exp)
                    if t >= 0:  # diagonal sub-block: triangular mask, both
                        # heads in one op (mask broadcast over the head dim)
                        nc.vector.tensor_tensor(
                            out=p3[:, :, t:t + P],
                            in0=p3[:, :, t:t + P],
                            in1=mask_sb.rearrange("p (o n) -> p o n", o=1).to_broadcast([P, 2, P]),
                            op=ALU.mult,
                        )
                    if fq:  # independent PE work to cover the exp latency
                        emit_fill(fq, 1)
                    for h2 in range(2):
                        nc.tensor.matmul(
                            av[h2][:, max(t, 0):NQ],
                            lhsT=vA[kb][:, 2 * mt + h2, :],
                            rhs=p3[:, h2, max(t, 0):NQ],
                            start=(kb == kb_lo),
                            stop=(kb == kb_hi - 1),
                        )
                for h2 in range(2):  # evict group into SBUF accumulator
                    if first:
                        nc.vector.tensor_copy(out=acc[h2], in_=av[h2])
                    else:
                        nc.vector.tensor_tensor(
                            out=acc[h2], in0=acc[h2], in1=av[h2], op=ALU.add,
                        )

            # ---- softmax normalize: 1/den broadcast + multiply ----
            def fin_block(mt, qc, acc, j):
                """On-chip normalize of one 128-query block (both heads)."""
                sl = slice(j * P, (j + 1) * P)
                for h2 in range(2):
                    rec = small_pool.tile([D_K, P], F32, tag="recb",
                                          name=f"recb{mt}_{qc}_{h2}_{j}")
                    nc.vector.reciprocal(rec, acc[h2][D_K:2 * D_K, sl])
                    nc.vector.tensor_tensor(
                        out=aoTq[mt][qc][h2 * D_K:(h2 + 1) * D_K, sl],
                        in0=acc[h2][0:D_K, sl],
                        in1=rec,
                        op=ALU.mult,
                    )

            def attn_finish(mt, qc, acc):
                den_d, den4, rec4, rec_d, bc = {}, {}, {}, {}, {}
                for h2 in range(2):
                    den_d[h2] = dr_pool.tile([1, NQ], F32, tag="dend",
                                             name=f"dend{mt}_{qc}_{h2}")
                    nc.sync.dma_start(out=den_d[h2],
                                      in_=acc[h2][D_K:D_K + 1, :])
                for h2 in range(2):
                    den4[h2] = small_pool.tile([P, NQ // P], F32, tag="den4",
                                               name=f"den4{mt}_{qc}_{h2}")
                    nc.sync.dma_start(
                        out=den4[h2],
                        in_=den_d[h2].rearrange("one (p f) -> (one p) f", p=P))
                for h2 in range(2):
                    rec4[h2] = small_pool.tile([P, NQ // P], F32, tag="rec4",
                                               name=f"rec4{mt}_{qc}_{h2}")
                    nc.vector.reciprocal(rec4[h2], den4[h2])
                for h2 in range(2):
                    rec_d[h2] = dr_pool.tile([1, NQ], F32, tag="recd",
                                             name=f"recd{mt}_{qc}_{h2}")
                    nc.sync.dma_start(
                        out=rec_d[h2].rearrange("one (p f) -> (one p) f", p=P),
                        in_=rec4[h2])
                for h2 in range(2):
                    bc[h2] = small_pool.tile([D_K, NQ], F32, tag="bc",
                                             name=f"bc{mt}_{qc}_{h2}")
                    nc.sync.dma_start(out=bc[h2],
                                      in_=rec_d[h2].to_broadcast([D_K, NQ]))
                for h2 in range(2):
                    nc.vector.tensor_tensor(
                        out=aoTq[mt][qc][h2 * D_K:(h2 + 1) * D_K, :],
                        in0=acc[h2][0:D_K, :],
                        in1=bc[h2],
                        op=ALU.mult,
                    )

            # ---- O projection: one 128-token row block x one 512-col block;
            # stores rotate between the sync and gpsimd queues ----
            def o_frag(qc, j, dc):
                def f():
                    lt = 4 * qc + j
                    ps = ps_mm.tile([P, NQ], F32, tag="mm", name=f"po{lt}_{dc}")
                    for kt in range(N_MT):
                        nc.tensor.matmul(
                            ps,
                            lhsT=aoTq[kt][qc][:, j * P:(j + 1) * P],
                            rhs=wo_sb[:, kt, dc * NQ:(dc + 1) * NQ],
                            start=(kt == 0),
                            stop=(kt == N_MT - 1),
                        )
                    ot = osb_pool.tile([P, NQ], BF16, tag="ot", name=f"ot{lt}_{dc}")
                    nc.vector.tensor_copy(out=ot, in_=ps)
                    eng = nc.sync if (dc == 0 or qc == 3) else nc.gpsimd
                    eng.dma_start(
                        out=out[lt * P:(lt + 1) * P, dc * NQ:(dc + 1) * NQ],
                        in_=ot,
                    )
                return f

            def o_frag3_split(j, dc, pool="mm"):
                cell = {}

                def f1():
                    lt = 12 + j
                    if pool == "mm":
                        ps = ps_mm.tile([P, NQ], F32, tag="mm",
                                        name=f"po{lt}_{dc}")
                    else:
                        big = ps_s.tile([P, 2 * NQ], F32, tag="s",
                                        name=f"po{lt}_{dc}")
                        ps = big[:, 0:NQ]
                    cell["ps"] = ps
                    for kt in range(3):
                        nc.tensor.matmul(
                            ps,
                            lhsT=aoTq[kt][3][:, j * P:(j + 1) * P],
                            rhs=wo_sb[:, kt, dc * NQ:(dc + 1) * NQ],
                            start=(kt == 0),
                            stop=False,
                        )

                def f2():
                    lt = 12 + j
                    ps = cell["ps"]
                    nc.tensor.matmul(
                        ps,
                        lhsT=aoTq[3][3][:, j * P:(j + 1) * P],
                        rhs=wo_sb[:, 3, dc * NQ:(dc + 1) * NQ],
                        start=False,
                        stop=True,
                    )
                    ot = osb_pool.tile([P, NQ], BF16, tag="ot", name=f"ot{lt}_{dc}")
                    if dc == 0:
                        nc.scalar.copy(out=ot, in_=ps)
                    else:
                        nc.vector.tensor_copy(out=ot, in_=ps)
                    nc.sync.dma_start(
                        out=out[lt * P:(lt + 1) * P, dc * NQ:(dc + 1) * NQ],
                        in_=ot,
                    )
                return f1, f2

            # ================= emission schedule =================
            # prologue: k0 / v0 / q0 projections; weight loads staged behind
            kq_dmas("k", xkT, 0, split=True)
            bk_sb = const.tile([P, N_MT], F32, tag="bk")
            nc.scalar.dma_start(out=bk_sb, in_=bk[:, :])
            bq_sb = const.tile([P, N_MT], F32, tag="bq")
            nc.scalar.dma_start(out=bq_sb, in_=bq[:, :])
            mask_sb = const.tile([P, P], BF16, tag="mask")
            nc.scalar.dma_start(out=mask_sb, in_=maskc[:, :])
            bv_sb = const.tile([P, GD], F32, tag="bv")
            nc.sync.dma_start(out=bv_sb, in_=bv[:, :].to_broadcast([P, GD]))
            xv_dmas(0)
            for mt in range(N_MT):
                kq_part(wk_sb, bk_sb, kTt, 1.0, "k", 0, mt)
            late_loads()
            for tt in range(4):
                v_proj_tile(tt)
            kq_dmas("q", xqT, 0, split=True)
            kq_dmas("k", xkT, 1)
            xv_dmas(1, both_gpsimd=True)
            kq_dmas("q", xqT, 3, split=True)
            for mt in range(N_MT):
                kq_part(wq_sb, bq_sb, qTt, 1.0, "q", 0, mt)

            cur_acc = {}

            def new_acc(mt, qc):
                a = [acc_pool.tile([P, NQ], F32, tag="acc",
                                   name=f"acc{mt}_{qc}_{i}") for i in range(2)]
                cur_acc[(mt, qc)] = a
                return a

            # wave 0: qc=0 attention; fill = k1/v1/q1/q3 projections
            fq = []
            for mt in range(N_MT):
                fq += kq_frags(wk_sb, bk_sb, kTt, 1.0, "k", 1, mt)
            for tt in range(4, 8):
                fq += v_frags(tt)
            for mt in range(N_MT):
                fq += kq_frags(wq_sb, bq_sb, qTt, 1.0, "q", 1, mt)
            for mt in range(N_MT):
                fq += kq_frags(wq_sb, bq_sb, qTt, 1.0, "q", 3, mt)
            for mt in range(N_MT):
                if mt == 0:
                    kq_dmas("q", xqT, 1)
                if mt == 2:
                    kq_dmas("k", xkT, 2)
                if mt == 3:
                    xv_dmas(2)
                attn_group(mt, 0, 0, 4, new_acc(mt, 0), first=True, fq=fq)
                attn_finish(mt, 0, cur_acc[(mt, 0)])
                emit_fill(fq, 4)
            emit_fill(fq, len(fq))

            # wave 1: qc=1 + pulled-forward qc=3 kc=0; fill = k2/v2/q2 + op0
            fq = []
            for mt in range(N_MT):
                fq += kq_frags(wk_sb, bk_sb, kTt, 1.0, "k", 2, mt)
            for tt in range(8, 12):
                fq += v_frags(tt)
            for mt in range(N_MT):
                fq += kq_frags(wq_sb, bq_sb, qTt, 1.0, "q", 2, mt)
            for j in range(4):
                fq += [o_frag(0, j, 0), o_frag(0, j, 1)]
            for mt in range(N_MT):
                if mt == 0:
                    kq_dmas("q", xqT, 2)
                if mt == 2:
                    kq_dmas("k", xkT, 3)
                if mt == 3:
                    xv_dmas(3)
                attn_group(mt, 1, 0, 8, new_acc(mt, 1), first=True, fq=fq)
                attn_finish(mt, 1, cur_acc[(mt, 1)])
                attn_group(mt, 3, 0, 4, acc3[mt], first=True, fq=fq)
                emit_fill(fq, 2)
            emit_fill(fq, len(fq))

            # wave 2: qc=2; fill = k3/v3 + op1
            fq = []
            for mt in range(N_MT):
                fq += kq_frags(wk_sb, bk_sb, kTt, 1.0, "k", 3, mt, act=False)
            for tt in range(12, 16):
                fq += v_frags(tt)
            for j in range(4):
                fq += [o_frag(1, j, 0), o_frag(1, j, 1)]
            for mt in range(N_MT):
                attn_group(mt, 2, 0, 8, new_acc(mt, 2), first=True, fq=fq)
                attn_group(mt, 2, 8, 12, cur_acc[(mt, 2)], first=False, fq=fq)
                attn_finish(mt, 2, cur_acc[(mt, 2)])
                emit_fill(fq, 2)
            emit_fill(fq, len(fq))

            # wave 3: qc=3 remaining groups, finishes spread per-mt; o_proj
            # chunk-3 chains split so kt:0-2 runs as fill during the mt=3
            # attention (normalizes 0-2 are done) and only the kt=3 step
            # trails the last normalize.  Chains borrow the idle score-PSUM
            # pool so four can be in flight.
            fq = []
            for j in range(4):
                fq += [o_frag(2, j, 0), o_frag(2, j, 1)]
            pools = ["mm", "mm", "s", "s"]
            op3 = [o_frag3_split(j, dc, pools[(2 * j + dc) % 4])
                   for j in range(4) for dc in range(2)]
            for mt in range(3):
                attn_group(mt, 3, 4, 12, acc3[mt], first=False, fq=fq)
                attn_group(mt, 3, 12, 16, acc3[mt], first=False, fq=fq)
                attn_finish(mt, 3, acc3[mt])
                emit_fill(fq, 2)
            fq3 = [op3[0][0], op3[1][0]]
            attn_group(3, 3, 4, 12, acc3[3], first=False, fq=fq)
            attn_group(3, 3, 12, 16, acc3[3], first=False, fq=fq3)
            emit_fill(fq, len(fq))
            emit_fill(fq3, len(fq3))
            op3[2][0]()
            op3[3][0]()
            fin_block(3, 3, acc3[3], 0)
            op3[0][1]()
            op3[1][1]()
            op3[4][0]()
            op3[5][0]()
            fin_block(3, 3, acc3[3], 1)
            op3[2][1]()
            op3[3][1]()
            op3[6][0]()
            op3[7][0]()
            fin_block(3, 3, acc3[3], 2)
            op3[4][1]()
            op3[5][1]()
            fin_block(3, 3, acc3[3], 3)
            op3[6][1]()
            op3[7][1]()
    nc.finalize()
    return nc


def make_in_maps(Q, K, V, Wq, bq, Wk, bk, Wv, bv, Wo, bo, attn_mask=None):
    """Build the 8 per-core input maps from full (unsharded) inputs."""
    Q = np.asarray(Q, np.float32)
    K = np.asarray(K, np.float32)
    V = np.asarray(V, np.float32)
    Wq = np.asarray(Wq, np.float32)
    Wk = np.asarray(Wk, np.float32)
    Wv = np.asarray(Wv, np.float32)
    Wo = np.asarray(Wo, np.float32)
    bq = np.asarray(bq, np.float32)
    bk = np.asarray(bk, np.float32)
    bv = np.asarray(bv, np.float32)

    i_idx = np.arange(P)[:, None]
    j_idx = np.arange(P)[None, :]
    maskc = (i_idx <= j_idx).astype(NPBF16)

    xT = {}
    for b in range(B):
        xT[b] = tuple(
            np.ascontiguousarray(X[b].T).astype(NPBF16) for X in (Q, K, V)
        )
    grp = {}
    for g in range(TP):
        sl = slice(g * GD, (g + 1) * GD)
        s_qk = np.float32(1.0 / np.sqrt(np.float32(D_K)))
        grp[g] = dict(
            wqT=np.ascontiguousarray(Wq[sl, :].T * s_qk).astype(NPBF16),
            wkT=np.ascontiguousarray(Wk[sl, :].T).astype(NPBF16),
            wvT=np.ascontiguousarray(Wv[sl, :].T).astype(NPBF16),
            woT=np.ascontiguousarray(Wo[:, sl].T).astype(NPBF16),
            bq=np.ascontiguousarray((bq[sl] * s_qk).reshape(N_MT, P).T).astype(np.float32),
            bk=np.ascontiguousarray(bk[sl].reshape(N_MT, P).T).astype(np.float32),
            bv=np.ascontiguousarray(bv[sl].reshape(1, GD)).astype(np.float32),
        )
    in_maps = []
    for c in range(2 * B):
        b, g = c // 2, c % 2
        m = dict(grp[g])
        m["xqT"], m["xkT"], m["xvT"] = xT[b]
        m["maskc"] = maskc
        in_maps.append(m)
    return in_maps


def assemble_output(results, bo):
    bo = np.asarray(bo, np.float32)
    out = np.empty((B, L, D_MODEL), np.float32)
    for b in range(B):
        out[b] = (results[2 * b]["out"].astype(np.float32)
                  + results[2 * b + 1]["out"].astype(np.float32) + bo)
    return out


_NC_CACHE = None


def kernel(**inputs) -> np.ndarray:
    global _NC_CACHE
    from concourse.bass_utils import run_bass_kernel_spmd

    if _NC_CACHE is None:
        _NC_CACHE = build_nc()
    in_maps = make_in_maps(**inputs)
    res = run_bass_kernel_spmd(_NC_CACHE, in_maps, core_ids=list(range(2 * B)))
    return assemble_output(res.results, inputs["bo"])


# revision 42
# speedup vs baseline: 1.0678x; 1.0678x over previous
"""Multi-head causal attention (QKV proj + attention + O proj) on 8 TRN2 cores.

Sharding: data-parallel over batch (4) x tensor-parallel over heads (2 groups
of 8 heads).  Core c handles batch c//2, head-group c%2.  Each core computes
its group's partial o_proj output; the host sums the two partials per batch.

Layout strategy (all activations arrive pre-transposed from the host, so the
kernel never transposes on-device):
  - qT, kT per head-pair M-tile: (128 head-dims, L) from  W.T-slice @ X.T
  - v natural (tokens, head-dims) with a fused ones-column for the softmax
    denominator: av_psum = v_aug.T @ P.T gives (65, Nq) where row 64 is the
    per-query sum of probabilities.
  - scores are computed transposed (keys on partitions, queries free), exp is
    taken without max-subtraction (scores are O(+-6) here, exp is safe in
    fp32), causal masking multiplies a small triangular mask after exp; fully
    masked column ranges are skipped by exp.

Schedule: the scalar engine (exp) is the scarce resource in the attention
phase and its natural per-query-chunk load grows linearly with the causal
width (18/37/55/73 us per wave).  To flatten it, query-chunk 3's first
key-group is pulled forward into wave 1 (its K/V/Q inputs are ready by
then), giving per-wave exp loads of 18/55/55/55 us.  Because the tensor
engine executes matmuls strictly in order, projection chains are split into
4-matmul fragments and a fill queue pops one fragment between each score
pair and its exp-dependent AV matmul, so the ~1 us exp latency is always
covered by queued independent work.  o_proj for chunk c is emitted during
wave c+1 as fill; chunk 3's chains are split around the last softmax
normalize (kt 0-2 before, kt 3 after) and borrow the then-idle score-PSUM
pool so four chains are in flight.  The last normalize runs on-chip per
128-query block (reciprocal of denominator rows that an extra block of 64
ones-columns in the V tiles replicates across partitions); all other
normalizes use a DMA transpose through DRAM so the reciprocal runs at 4
elements/lane.  Inputs load as two-half 3D-AP DMAs (region-tracked, so
consumers start after the first half).  Outputs are stored bf16.
Compute dtype bf16 (fp32 PSUM accumulation).
"""

import numpy as np
import ml_dtypes

import concourse.bass as bass
import concourse.tile as tile
from concourse import bacc, mybir

D_MODEL = 1024
N_HEADS = 16
D_K = 64
B, L = 4, 2048
TP = 2                  # head groups
GD = D_MODEL // TP      # 512 head-dims per group
P = 128
NQ = 512                # query chunk (one fp32 PSUM bank)
N_MT = GD // P          # 4 M-tiles (head pairs) per group
N_KT = D_MODEL // P     # 8 contraction tiles over model dim
N_TT = L // P           # 16 token tiles
N_QC = L // NQ          # 4 query chunks
BF16 = mybir.dt.bfloat16
F32 = mybir.dt.float32
NPBF16 = ml_dtypes.bfloat16
AF = mybir.ActivationFunctionType
ALU = mybir.AluOpType


def build_nc() -> bass.Bass:
    nc = bacc.Bacc("TRN2", target_bir_lowering=False)

    xqT = nc.dram_tensor("xqT", [D_MODEL, L], BF16, kind="ExternalInput")
    xkT = nc.dram_tensor("xkT", [D_MODEL, L], BF16, kind="ExternalInput")
    xvT = nc.dram_tensor("xvT", [D_MODEL, L], BF16, kind="ExternalInput")
    wqT = nc.dram_tensor("wqT", [D_MODEL, GD], BF16, kind="ExternalInput")
    wkT = nc.dram_tensor("wkT", [D_MODEL, GD], BF16, kind="ExternalInput")
    wvT = nc.dram_tensor("wvT", [D_MODEL, GD], BF16, kind="ExternalInput")
    woT = nc.dram_tensor("woT", [GD, D_MODEL], BF16, kind="ExternalInput")
    bq = nc.dram_tensor("bq", [P, N_MT], F32, kind="ExternalInput")
    bk = nc.dram_tensor("bk", [P, N_MT], F32, kind="ExternalInput")
    bv = nc.dram_tensor("bv", [1, GD], F32, kind="ExternalInput")
    maskc = nc.dram_tensor("maskc", [P, P], BF16, kind="ExternalInput")
    out = nc.dram_tensor("out", [L, D_MODEL], BF16, kind="ExternalOutput")

    with tile.TileContext(nc) as tc:
        with (
            tc.tile_pool(name="const", bufs=1) as const,
            tc.tile_pool(name="xvg", bufs=2) as xvg_pool,
            tc.tile_pool(name="xch", bufs=4) as xch_pool,
            tc.tile_pool(name="pt", bufs=4) as pt_pool,
            tc.tile_pool(name="small", bufs=3) as small_pool,
            tc.tile_pool(name="acc", bufs=4) as acc_pool,
            tc.tile_pool(name="osb", bufs=4) as osb_pool,
            tc.tile_pool(name="dr", bufs=4, space="DRAM") as dr_pool,
            tc.tile_pool(name="ps_s", bufs=2, space="PSUM") as ps_s,
            tc.tile_pool(name="ps_av", bufs=2, space="PSUM") as ps_av,
            tc.tile_pool(name="ps_mm", bufs=2, space="PSUM") as ps_mm,
        ):
            # ---- resident constants / weights; first-needed tiles (wk + xk
            # chunk 0) round-robin over all three DMA queues
            QS = [nc.gpsimd, nc.scalar, nc.sync]
            wk_sb = const.tile([P, N_KT, GD], BF16, tag="wk")
            nc.gpsimd.dma_start(
                out=wk_sb[:, 0:4, :],
                in_=wkT.rearrange("(kt p) n -> p kt n", p=P)[:, 0:4, :])
            nc.scalar.dma_start(
                out=wk_sb[:, 4:8, :],
                in_=wkT.rearrange("(kt p) n -> p kt n", p=P)[:, 4:8, :])

            # per-(mt, chunk) tiles so consumers unblock as soon as possible
            qTt = [[const.tile([P, NQ], BF16, tag=f"qT{mt}_{ncz}", name=f"qT{mt}_{ncz}")
                    for ncz in range(N_QC)] for mt in range(N_MT)]
            kTt = [[const.tile([P, NQ], BF16, tag=f"kT{mt}_{ncz}", name=f"kT{mt}_{ncz}")
                    for ncz in range(N_QC)] for mt in range(N_MT)]
            vA = [const.tile([P, 2 * N_MT, 2 * D_K], BF16, tag=f"v{tt}", name=f"v{tt}")
                  for tt in range(N_TT)]
            aoTq = [[const.tile([P, NQ], BF16, tag=f"ao{mt}_{qc}", name=f"ao{mt}_{qc}")
                     for qc in range(N_QC)] for mt in range(N_MT)]
            # persistent accumulators for the pulled-forward qc=3 groups
            acc3 = [[const.tile([P, NQ], F32, tag=f"acc3_{mt}_{h2}",
                                name=f"acc3_{mt}_{h2}") for h2 in range(2)]
                    for mt in range(N_MT)]

            scale = float(1.0 / np.sqrt(np.float32(D_K)))

            wv_sb = const.tile([P, N_KT, GD], BF16, tag="wv")
            wq_sb = const.tile([P, N_KT, GD], BF16, tag="wq")
            wo_sb = const.tile([P, N_MT, D_MODEL], BF16, tag="wo")

            def late_loads():
                # per-kt loads so the scalar queue is clear before exps start;
                # wo (needed only in wave 1) rides gpsimd/sync
                for kt in range(N_KT):
                    nc.scalar.dma_start(out=wv_sb[:, kt, :],
                                        in_=wvT[kt * P:(kt + 1) * P, :])
                for kt in range(N_KT):
                    nc.scalar.dma_start(out=wq_sb[:, kt, :],
                                        in_=wqT[kt * P:(kt + 1) * P, :])
                for kt in range(N_MT):
                    eng = nc.gpsimd if kt % 2 == 0 else nc.sync
                    eng.dma_start(out=wo_sb[:, kt, :],
                                  in_=woT[kt * P:(kt + 1) * P, :])

            # ---- V projection: per-group (4 token tiles) input chunks
            xvg_cache = {}

            def xv_dmas(g, both_gpsimd=False):
                xc = xvg_pool.tile([P, N_KT, 4 * P], BF16, tag="xvg",
                                   name=f"xv{g}")
                src = xvT.rearrange("(kt p) n -> p kt n", p=P)
                e1 = nc.gpsimd if both_gpsimd else nc.sync
                nc.gpsimd.dma_start(
                    out=xc[:, 0:4, :],
                    in_=src[:, 0:4, g * 4 * P:(g + 1) * 4 * P])
                e1.dma_start(
                    out=xc[:, 4:8, :],
                    in_=src[:, 4:8, g * 4 * P:(g + 1) * 4 * P])
                xvg_cache[g] = xc

            def v_frags(tt):
                g, j = tt // 4, tt % 4
                cell = {}

                def f1():
                    xvs = [xvg_cache[g][:, kt, :] for kt in range(N_KT)]
                    ps = cell["ps"] = ps_mm.tile([P, GD], F32, tag="mm",
                                                 name=f"psv{tt}")
                    for kt in range(4):
                        nc.tensor.matmul(
                            ps, lhsT=xvs[kt][:, j * P:(j + 1) * P],
                            rhs=wv_sb[:, kt, :], start=(kt == 0), stop=False)

                def f2():
                    xvs = [xvg_cache[g][:, kt, :] for kt in range(N_KT)]
                    ps = cell["ps"]
                    for kt in range(4, N_KT):
                        nc.tensor.matmul(
                            ps, lhsT=xvs[kt][:, j * P:(j + 1) * P],
                            rhs=wv_sb[:, kt, :], start=False,
                            stop=(kt == N_KT - 1))
                    nc.vector.tensor_tensor(
                        out=vA[tt][:, :, 0:D_K],
                        in0=ps.rearrange("p (h d) -> p h d", d=D_K),
                        in1=bv_sb.rearrange("p (h d) -> p h d", d=D_K),
                        op=ALU.add,
                    )
                    nc.vector.memset(vA[tt][:, :, D_K:2 * D_K], 1.0)
                return [f1, f2]

            def v_proj_tile(tt):
                for f in v_frags(tt):
                    f()

            xch_cache = {}

            def kq_dmas(nm, x_dram, ncz, split=False):
                xc = xch_pool.tile([P, N_KT, NQ], BF16, tag="xch",
                                   name=f"x{nm}{ncz}")
                e0, e1 = (nc.sync, nc.gpsimd) if split else (nc.gpsimd, nc.sync)
                src = x_dram.rearrange("(kt p) n -> p kt n", p=P)
                e0.dma_start(
                    out=xc[:, 0:4, :],
                    in_=src[:, 0:4, ncz * NQ:(ncz + 1) * NQ])
                e1.dma_start(
                    out=xc[:, 4:8, :],
                    in_=src[:, 4:8, ncz * NQ:(ncz + 1) * NQ])
                xch_cache[(nm, ncz)] = xc

            def kq_frags(w_sb, b_sb, dsts, sc, nm, ncz, mt, act=False):
                cell = {}

                def f1():
                    xch = xch_cache[(nm, ncz)]
                    ps = cell["ps"] = ps_mm.tile([P, NQ], F32, tag="mm",
                                                 name=f"ps{nm}{ncz}{mt}")
                    for kt in range(4):
                        nc.tensor.matmul(
                            ps, lhsT=w_sb[:, kt, mt * P:(mt + 1) * P],
                            rhs=xch[:, kt, :], start=(kt == 0), stop=False)

                def f2():
                    xch = xch_cache[(nm, ncz)]
                    ps = cell["ps"]
                    for kt in range(4, N_KT):
                        nc.tensor.matmul(
                            ps, lhsT=w_sb[:, kt, mt * P:(mt + 1) * P],
                            rhs=xch[:, kt, :], start=False,
                            stop=(kt == N_KT - 1))
                    if act:  # ACT has slack outside wave 2; keeps DVE clear
                        nc.scalar.add(out=dsts[mt][ncz], in_=ps,
                                      add=b_sb[:, mt:mt + 1])
                    else:
                        nc.vector.tensor_scalar(
                            out=dsts[mt][ncz],
                            in0=ps,
                            scalar1=b_sb[:, mt:mt + 1],
                            scalar2=sc,
                            op0=ALU.add,
                            op1=ALU.mult,
                        )
                return [f1, f2]

            def kq_part(w_sb, b_sb, dsts, sc, nm, ncz, mt):
                for f in kq_frags(w_sb, b_sb, dsts, sc, nm, ncz, mt):
                    f()

            # ---- attention: one 4-key-block group (512 keys) of (mt, qc) ----
            def emit_fill(fq, n=1):
                for _ in range(n):
                    if fq:
                        fq.pop(0)()

            def attn_group(mt, qc, kb_lo, kb_hi, acc, first, fq=None):
                av = [ps_av.tile([P, NQ], F32, tag="av",
                                 name=f"av{mt}_{qc}_{kb_lo}_{i}") for i in range(2)]
                for kb in range(kb_lo, kb_hi):
                    t = P * (kb - 4 * qc)  # <0 for full blocks
                    s_ps = ps_s.tile([P, 2 * NQ], F32, tag="s",
                                     name=f"s{mt}_{qc}_{kb}")
                    s3 = s_ps.rearrange("p (h n) -> p h n", n=NQ)
                    for h2 in range(2):
                        nc.tensor.matmul(
                            s3[:, h2, max(t, 0):NQ],
                            lhsT=kTt[mt][kb // 4][h2 * D_K:(h2 + 1) * D_K,
                                                 (kb % 4) * P:(kb % 4 + 1) * P],
                            rhs=qTt[mt][qc][h2 * D_K:(h2 + 1) * D_K,
                                            max(t, 0):NQ],
                            start=True,
                            stop=True,
                        )
                    pt = pt_pool.tile([P, 2 * NQ], BF16, tag="pt",
                                      name=f"pt{mt}_{qc}_{kb}")
                    p3 = pt.rearrange("p (h n) -> p h n", n=NQ)
                    if t <= 0:
                        nc.scalar.activation(out=pt, in_=s_ps, func=AF.Exp)
                    else:
                        nc.scalar.activation(out=p3[:, :, t:NQ],
                                             in_=s3[:, :, t:NQ], func=AF.Exp)
                    if t >= 0:  # diagonal sub-block: triangular mask, both
                        # heads in one op (mask broadcast over the head dim)
                        nc.vector.tensor_tensor(
                            out=p3[:, :, t:t + P],
                            in0=p3[:, :, t:t + P],
                            in1=mask_sb.rearrange("p (o n) -> p o n", o=1).to_broadcast([P, 2, P]),
                            op=ALU.mult,
                        )
                    if fq:  # independent PE work to cover the exp latency
                        emit_fill(fq, 1)
                    for h2 in range(2):
                        nc.tensor.matmul(
                            av[h2][:, max(t, 0):NQ],
                            lhsT=vA[kb][:, 2 * mt + h2, :],
                            rhs=p3[:, h2, max(t, 0):NQ],
                            start=(kb == kb_lo),
                            stop=(kb == kb_hi - 1),
                        )
                for h2 in range(2):  # evict group into SBUF accumulator
                    if first:
                        nc.vector.tensor_copy(out=acc[h2], in_=av[h2])
                    else:
                        nc.vector.tensor_tensor(
                            out=acc[h2], in0=acc[h2], in1=av[h2], op=ALU.add,
                        )

            # ---- softmax normalize: 1/den broadcast + multiply ----
            def fin_block(mt, qc, acc, j):
                """On-chip normalize of one 128-query block (both heads)."""
                sl = slice(j * P, (j + 1) * P)
                for h2 in range(2):
                    rec = small_pool.tile([D_K, P], F32, tag="recb",
                                          name=f"recb{mt}_{qc}_{h2}_{j}")
                    nc.vector.reciprocal(rec, acc[h2][D_K:2 * D_K, sl])
                    nc.vector.tensor_tensor(
                        out=aoTq[mt][qc][h2 * D_K:(h2 + 1) * D_K, sl],
                        in0=acc[h2][0:D_K, sl],
                        in1=rec,
                        op=ALU.mult,
                    )

            def attn_finish(mt, qc, acc):
                den_d, den4, rec4, rec_d, bc = {}, {}, {}, {}, {}
                for h2 in range(2):
                    den_d[h2] = dr_pool.tile([1, NQ], F32, tag="dend",
                                             name=f"dend{mt}_{qc}_{h2}")
                    nc.sync.dma_start(out=den_d[h2],
                                      in_=acc[h2][D_K:D_K + 1, :])
                for h2 in range(2):
                    den4[h2] = small_pool.tile([P, NQ // P], F32, tag="den4",
                                               name=f"den4{mt}_{qc}_{h2}")
                    nc.sync.dma_start(
                        out=den4[h2],
                        in_=den_d[h2].rearrange("one (p f) -> (one p) f", p=P))
                for h2 in range(2):
                    rec4[h2] = small_pool.tile([P, NQ // P], F32, tag="rec4",
                                               name=f"rec4{mt}_{qc}_{h2}")
                    nc.vector.reciprocal(rec4[h2], den4[h2])
                for h2 in range(2):
                    rec_d[h2] = dr_pool.tile([1, NQ], F32, tag="recd",
                                             name=f"recd{mt}_{qc}_{h2}")
                    nc.sync.dma_start(
                        out=rec_d[h2].rearrange("one (p f) -> (one p) f", p=P),
                        in_=rec4[h2])
                for h2 in range(2):
                    bc[h2] = small_pool.tile([D_K, NQ], F32, tag="bc",
                                             name=f"bc{mt}_{qc}_{h2}")
                    nc.sync.dma_start(out=bc[h2],
                                      in_=rec_d[h2].to_broadcast([D_K, NQ]))
                for h2 in range(2):
                    nc.vector.tensor_tensor(
                        out=aoTq[mt][qc][h2 * D_K:(h2 + 1) * D_K, :],
                        in0=acc[h2][0:D_K, :],
                        in1=bc[h2],
                        op=ALU.mult,
                    )

            # ---- O projection: one 128-token row block x one 512-col block;
            # stores rotate between the sync and gpsimd queues ----
            def o_frag(qc, j, dc):
                def f():
                    lt = 4 * qc + j
                    ps = ps_mm.tile([P, NQ], F32, tag="mm", name=f"po{lt}_{dc}")
                    for kt in range(N_MT):
                        nc.tensor.matmul(
                            ps,
                            lhsT=aoTq[kt][qc][:, j * P:(j + 1) * P],
                            rhs=wo_sb[:, kt, dc * NQ:(dc + 1) * NQ],
                            start=(kt == 0),
                            stop=(kt == N_MT - 1),
                        )
                    ot = osb_pool.tile([P, NQ], BF16, tag="ot", name=f"ot{lt}_{dc}")
                    nc.vector.tensor_copy(out=ot, in_=ps)
                    eng = nc.sync if (dc == 0 or qc == 3) else nc.gpsimd
                    eng.dma_start(
                        out=out[lt * P:(lt + 1) * P, dc * NQ:(dc + 1) * NQ],
                        in_=ot,
                    )
                return f

            def o_frag3_split(j, dc, pool="mm"):
                cell = {}

                def f1():
                    lt = 12 + j
                    if pool == "mm":
                        ps = ps_mm.tile([P, NQ], F32, tag="mm",
                                        name=f"po{lt}_{dc}")
                    else:
                        big = ps_s.tile([P, 2 * NQ], F32, tag="s",
                                        name=f"po{lt}_{dc}")
                        ps = big[:, 0:NQ]
                    cell["ps"] = ps
                    for kt in range(3):
                        nc.tensor.matmul(
                            ps,
                            lhsT=aoTq[kt][3][:, j * P:(j + 1) * P],
                            rhs=wo_sb[:, kt, dc * NQ:(dc + 1) * NQ],
                            start=(kt == 0),
                            stop=False,
                        )

                def f2():
                    lt = 12 + j
                    ps = cell["ps"]
                    nc.tensor.matmul(
                        ps,
                        lhsT=aoTq[3][3][:, j * P:(j + 1) * P],
                        rhs=wo_sb[:, 3, dc * NQ:(dc + 1) * NQ],
                        start=False,
                        stop=True,
                    )
                    ot = osb_pool.tile([P, NQ], BF16, tag="ot", name=f"ot{lt}_{dc}")
                    if dc == 0:
                        nc.scalar.copy(out=ot, in_=ps)
                    else:
                        nc.vector.tensor_copy(out=ot, in_=ps)
                    nc.sync.dma_start(
                        out=out[lt * P:(lt + 1) * P, dc * NQ:(dc + 1) * NQ],
                        in_=ot,
                    )
                return f1, f2

            # ================= emission schedule =================
            # prologue: k0 / v0 / q0 projections; weight loads staged behind
            kq_dmas("k", xkT, 0, split=True)
            bk_sb = const.tile([P, N_MT], F32, tag="bk")
            nc.sync.dma_start(out=bk_sb, in_=bk[:, :])
            bq_sb = const.tile([P, N_MT], F32, tag="bq")
            nc.sync.dma_start(out=bq_sb, in_=bq[:, :])
            bv_sb = const.tile([P, GD], F32, tag="bv")
            nc.sync.dma_start(out=bv_sb, in_=bv[:, :].to_broadcast([P, GD]))
            mask_sb = const.tile([P, P], BF16, tag="mask")
            nc.sync.dma_start(out=mask_sb, in_=maskc[:, :])
            xv_dmas(0)
            for mt in range(N_MT):
                kq_part(wk_sb, bk_sb, kTt, 1.0, "k", 0, mt)
            late_loads()
            for tt in range(4):
                v_proj_tile(tt)
            kq_dmas("q", xqT, 0, split=True)
            kq_dmas("k", xkT, 1)
            xv_dmas(1, both_gpsimd=True)
            kq_dmas("q", xqT, 3, split=True)
            for mt in range(N_MT):
                kq_part(wq_sb, bq_sb, qTt, 1.0, "q", 0, mt)

            cur_acc = {}

            def new_acc(mt, qc):
                a = [acc_pool.tile([P, NQ], F32, tag="acc",
                                   name=f"acc{mt}_{qc}_{i}") for i in range(2)]
                cur_acc[(mt, qc)] = a
                return a

            # wave 0: qc=0 attention; fill = k1/v1/q1/q3 projections
            fq = []
            for mt in range(N_MT):
                fq += kq_frags(wk_sb, bk_sb, kTt, 1.0, "k", 1, mt)
            for tt in range(4, 8):
                fq += v_frags(tt)
            for mt in range(N_MT):
                fq += kq_frags(wq_sb, bq_sb, qTt, 1.0, "q", 1, mt)
            for mt in range(N_MT):
                fq += kq_frags(wq_sb, bq_sb, qTt, 1.0, "q", 3, mt)
            for mt in range(N_MT):
                if mt == 0:
                    kq_dmas("q", xqT, 1)
                if mt == 2:
                    kq_dmas("k", xkT, 2)
                if mt == 3:
                    xv_dmas(2)
                attn_group(mt, 0, 0, 4, new_acc(mt, 0), first=True, fq=fq)
                attn_finish(mt, 0, cur_acc[(mt, 0)])
                emit_fill(fq, 4)
            emit_fill(fq, len(fq))

            # wave 1: qc=1 + pulled-forward qc=3 kc=0; fill = k2/v2/q2 + op0
            fq = []
            for mt in range(N_MT):
                fq += kq_frags(wk_sb, bk_sb, kTt, 1.0, "k", 2, mt)
            for tt in range(8, 12):
                fq += v_frags(tt)
            for mt in range(N_MT):
                fq += kq_frags(wq_sb, bq_sb, qTt, 1.0, "q", 2, mt)
            for j in range(4):
                fq += [o_frag(0, j, 0), o_frag(0, j, 1)]
            for mt in range(N_MT):
                if mt == 0:
                    kq_dmas("q", xqT, 2)
                if mt == 2:
                    kq_dmas("k", xkT, 3)
                if mt == 3:
                    xv_dmas(3)
                attn_group(mt, 1, 0, 8, new_acc(mt, 1), first=True, fq=fq)
                attn_finish(mt, 1, cur_acc[(mt, 1)])
                attn_group(mt, 3, 0, 4, acc3[mt], first=True, fq=fq)
                emit_fill(fq, 2)
            emit_fill(fq, len(fq))

            # wave 2: qc=2; fill = k3/v3 + op1
            fq = []
            for mt in range(N_MT):
                fq += kq_frags(wk_sb, bk_sb, kTt, 1.0, "k", 3, mt, act=False)
            for tt in range(12, 16):
                fq += v_frags(tt)
            for j in range(4):
                fq += [o_frag(1, j, 0), o_frag(1, j, 1)]
            for mt in range(N_MT):
                attn_group(mt, 2, 0, 8, new_acc(mt, 2), first=True, fq=fq)
                attn_group(mt, 2, 8, 12, cur_acc[(mt, 2)], first=False, fq=fq)
                attn_finish(mt, 2, cur_acc[(mt, 2)])
                emit_fill(fq, 2)
            emit_fill(fq, len(fq))

            # wave 3: qc=3 remaining groups, finishes spread per-mt; o_proj
            # chunk-3 chains split so kt:0-2 runs as fill during the mt=3
            # attention (normalizes 0-2 are done) and only the kt=3 step
            # trails the last normalize.  Chains borrow the idle score-PSUM
            # pool so four can be in flight.
            fq = []
            for j in range(4):
                fq += [o_frag(2, j, 0), o_frag(2, j, 1)]
            pools = ["mm", "mm", "s", "s"]
            op3 = [o_frag3_split(j, dc, pools[(2 * j + dc) % 4])
                   for j in range(4) for dc in range(2)]
            for mt in range(3):
                attn_group(mt, 3, 4, 12, acc3[mt], first=False, fq=fq)
                attn_group(mt, 3, 12, 16, acc3[mt], first=False, fq=fq)
                attn_finish(mt, 3, acc3[mt])
                emit_fill(fq, 2)
            fq3 = [op3[0][0], op3[1][0]]
            attn_group(3, 3, 4, 12, acc3[3], first=False, fq=fq)
            attn_group(3, 3, 12, 16, acc3[3], first=False, fq=fq3)
            emit_fill(fq, len(fq))
            emit_fill(fq3, len(fq3))
            op3[2][0]()
            op3[3][0]()
            fin_block(3, 3, acc3[3], 0)
            op3[0][1]()
            op3[1][1]()
            op3[4][0]()
            op3[5][0]()
            fin_block(3, 3, acc3[3], 1)
            op3[2][1]()
            op3[3][1]()
            op3[6][0]()
            op3[7][0]()
            fin_block(3, 3, acc3[3], 2)
            op3[4][1]()
            op3[5][1]()
            fin_block(3, 3, acc3[3], 3)
            op3[6][1]()
            op3[7][1]()
    nc.finalize()
    return nc


def make_in_maps(Q, K, V, Wq, bq, Wk, bk, Wv, bv, Wo, bo, attn_mask=None):
    """Build the 8 per-core input maps from full (unsharded) inputs."""
    Q = np.asarray(Q, np.float32)
    K = np.asarray(K, np.float32)
    V = np.asarray(V, np.float32)
    Wq = np.asarray(Wq, np.float32)
    Wk = np.asarray(Wk, np.float32)
    Wv = np.asarray(Wv, np.float32)
    Wo = np.asarray(Wo, np.float32)
    bq = np.asarray(bq, np.float32)
    bk = np.asarray(bk, np.float32)
    bv = np.asarray(bv, np.float32)

    i_idx = np.arange(P)[:, None]
    j_idx = np.arange(P)[None, :]
    maskc = (i_idx <= j_idx).astype(NPBF16)

    xT = {}
    for b in range(B):
        xT[b] = tuple(
            np.ascontiguousarray(X[b].T).astype(NPBF16) for X in (Q, K, V)
        )
    grp = {}
    for g in range(TP):
        sl = slice(g * GD, (g + 1) * GD)
        s_qk = np.float32(1.0 / np.sqrt(np.float32(D_K)))
        grp[g] = dict(
            wqT=np.ascontiguousarray(Wq[sl, :].T * s_qk).astype(NPBF16),
            wkT=np.ascontiguousarray(Wk[sl, :].T).astype(NPBF16),
            wvT=np.ascontiguousarray(Wv[sl, :].T).astype(NPBF16),
            woT=np.ascontiguousarray(Wo[:, sl].T).astype(NPBF16),
            bq=np.ascontiguousarray((bq[sl] * s_qk).reshape(N_MT, P).T).astype(np.float32),
            bk=np.ascontiguousarray(bk[sl].reshape(N_MT, P).T).astype(np.float32),
            bv=np.ascontiguousarray(bv[sl].reshape(1, GD)).astype(np.float32),
        )
    in_maps = []
    for c in range(2 * B):
        b, g = c // 2, c % 2
        m = dict(grp[g])
        m["xqT"], m["xkT"], m["xvT"] = xT[b]
        m["maskc"] = maskc
        in_maps.append(m)
    return in_maps


def assemble_output(results, bo):
    bo = np.asarray(bo, np.float32)
    out = np.empty((B, L, D_MODEL), np.float32)
    for b in range(B):
        out[b] = (results[2 * b]["out"].astype(np.float32)
                  + results[2 * b + 1]["out"].astype(np.float32) + bo)
    return out


_NC_CACHE = None


def kernel(**inputs) -> np.ndarray:
    global _NC_CACHE
    from concourse.bass_utils import run_bass_kernel_spmd

    if _NC_CACHE is None:
        _NC_CACHE = build_nc()
    in_maps = make_in_maps(**inputs)
    res = run_bass_kernel_spmd(_NC_CACHE, in_maps, core_ids=list(range(2 * B)))
    return assemble_output(res.results, inputs["bo"])
